# revision 1
# baseline (speedup 1.0000x reference)
"""COCOA loss kernel for 8 Trainium2 NeuronCores (v2).

loss = SCALE_LOSS * sum_b pos[b] + LAMBDA * sum(neg)
  pos[b] = mean_{v,w} exp((1 - zn[v,b]*zn[w,b]) / T)
  neg    = sum_{v,b,c!=b} exp(zn[v,b]*zn[v,c] / T) / (B-1)

v2 design (vs v1: 89.8us -> target ~55us):
  * Neg grams in fp8e4 with MatmulPerfMode.DoubleRow: k=256 contraction in
    one PE pass at 2x+ rate (HW-verified, rel err 1e-4; sim error budget
    ~2e-5 of the loss vs 2e-2 tolerance).
  * Optimal symmetric tile cover: per view the 32x32 grid of 128-tiles is
    covered with 528 tiles (vs 640 in v1): every row computes its diagonal
    tile (weight 1) plus offsets 1..16 (rows 0..15) or 1..15 (rows 16..31)
    at weight 2.  Each core gets rows {2c, 2c+1, 16+2c, 16+2c+1} per view
    = 66 tiles -> 50,688 exp-columns (ACT is the bottleneck engine; this
    is 17.5% less ACT work than v1).
  * Uniform-weight ACT groups: diagonal tiles get ln(0.5)/2 added via a
    k=1 rank-1 matmul into their PSUM region, so exp comes out halved and
    EVERY activation group is host-weighted x2 -> 25 big [128,2048]
    activation+accum instructions instead of 60+ small ones.
  * Pos term: self-pairs are analytic (=1); the 15 cross-view pair sims
    run as fused DVE scalar_tensor_tensor (mult+mult+accum) instructions.
  * True gram diagonal exp(2*|z|^2) ~ e^2 subtracted analytically on host.
"""

import sys

import numpy as np

try:
    import concourse.bass as bass  # noqa: F401
except ImportError:  # pragma: no cover
    sys.path.insert(0, "/opt/trn_rl_repo")

import concourse.bass as bass
import concourse.bacc as bacc
import concourse.mybir as mybir
import concourse.tile as tile
from concourse.bass_utils import run_bass_kernel_spmd

import ml_dtypes

BF16 = ml_dtypes.bfloat16
FP8NP = ml_dtypes.float8_e4m3

# Problem constants (hardcoded per the harness contract).
B = 4096
V = 6
D = 256
NCORE = 8
NT = B // 128          # 32 tile-rows per view
WIN = 2304             # per-row-pair local window (128 + 17*128)
EXT = B + 17 * 128     # wrap-extended column count = 6272

TEMPERATURE = 0.5
SCALE_LOSS = 1.0 / 32.0
LAMBDA = 0.0039

F32 = mybir.dt.float32
BF16_DT = mybir.dt.bfloat16
FP8_DT = mybir.dt.float8e4

GCOLS = 2048           # ACT group width (4 PSUM banks)
GTILES = GCOLS // 128  # 16 tile slots per group
NEG_TILES = 2 * 17 + 2 * 16          # 66 per view
NGROUP = (NEG_TILES * V + GTILES - 1) // GTILES   # 25
NSTAT = 48
POS_BASE = 32

C0 = float(np.log(0.5) / 2.0)        # bias that halves exp(2s)
E2 = float(np.exp(2.0))

# Pool/GPSIMD exp offload: the Q7 pow ucode is HW-correct from SBUF inputs
# but neuronx-cc cannot codegen it with a PSUM operand, and a PSUM->SBUF
# staging copy costs more than the offload saves.  Kept disabled.
POOL_SET = frozenset()

_PAIRS = [(v, w) for v in range(V) for w in range(v + 1, V)]  # 15

# Rows per core (local window coords are identical on every core):
# (src_idx 0=lo/1=hi, local col base of the row's stationary block, n_w2)
ROWS = [(0, 0, 16), (0, 128, 16), (1, 0, 15), (1, 128, 15)]


def _build_nc(reps: int = 1) -> bass.Bass:
    nc = bacc.Bacc("TRN2", debug=False, num_devices=NCORE)

    zlo_d = nc.dram_tensor("zlo", [V, 128, 2, WIN], FP8_DT, kind="ExternalInput")
    zhi_d = nc.dram_tensor("zhi", [V, 128, 2, WIN], FP8_DT, kind="ExternalInput")
    zb_d = nc.dram_tensor("zb", [4, 128, V * D], BF16_DT, kind="ExternalInput")
    st_d = nc.dram_tensor("stats", [128, NSTAT], F32, kind="ExternalOutput")

    with tile.TileContext(nc) as tc:
        with (
            tc.tile_pool(name="zp", bufs=1) as zp,
            tc.tile_pool(name="zbp", bufs=1) as zbp,
            tc.tile_pool(name="stp", bufs=1) as stp,
            tc.tile_pool(name="cstp", bufs=1) as cstp,
            tc.tile_pool(name="escp", bufs=2) as escp,
            tc.tile_pool(name="scrp", bufs=4) as scrp,
            tc.tile_pool(name="simp", bufs=4) as simp,
            tc.tile_pool(name="pexpp", bufs=4) as pexpp,
            tc.tile_pool(name="psp", bufs=2, space="PSUM") as psp,
        ):
            stats = stp.tile([128, NSTAT], F32)

            zsb = [[zp.tile([128, 2, WIN], FP8_DT, tag=f"z{s}_{v}",
                            name=f"z{s}_{v}") for v in range(V)]
                   for s in range(2)]
            zb_sb = [zbp.tile([128, V * D], BF16_DT, tag=f"zb_{t}",
                              name=f"zb_{t}") for t in range(4)]
            zd = [zlo_d, zhi_d]
            order = [(0, 0)] + [(s_, v) for v in range(V)
                                for s_ in range(2) if not (s_ == 0 and v == 0)]
            H = WIN // 2
            for (s_, v) in order[:2]:
                for h in range(2):
                    cs = slice(h * H, (h + 1) * H)
                    nc.sync.dma_start(zsb[s_][v][:, :, cs],
                                      zd[s_].ap()[v][:, :, cs])
            for t in range(4):
                nc.sync.dma_start(zb_sb[t][:, :], zb_d.ap()[t])
            for (s_, v) in order[2:]:
                for h in range(2):
                    cs = slice(h * H, (h + 1) * H)
                    nc.sync.dma_start(zsb[s_][v][:, :, cs],
                                      zd[s_].ap()[v][:, :, cs])

            # constants for the k=1 bias matmul (halve the diagonal tiles)
            bl = cstp.tile([1, 128], BF16_DT, tag="bl", name="bl")
            br = cstp.tile([1, 128], BF16_DT, tag="br", name="br")
            e2t = None
            nc.vector.memset(bl[:, :], C0)
            nc.vector.memset(br[:, :], 1.0)

            for _rep in range(reps):
                run_body(nc, zsb, zb_sb, bl, br, e2t, stats,
                         escp, scrp, simp, pexpp, psp)

            nc.sync.dma_start(st_d.ap()[:, :], stats[:, :])

    nc.compile()
    return nc


def run_body(nc, zsb, zb_sb, bl, br, e2t, stats, escp, scrp, simp, pexpp, psp):
    # ---- neg term: one global stream of 396 tiles chopped into 25
    # uniform PSUM groups; per group: fp8 DoubleRow matmuls + 1 ACT ----
    stream = []  # (view, src, row_base, tile_idx_in_row)
    for v in range(V):
        for (src, base, nw2) in ROWS:
            for t in range(nw2 + 1):
                stream.append((v, src, base, t))
    assert len(stream) == NEG_TILES * V

    # pos sims first: DVE fills all 4 m-tiles while ACT chews neg groups
    sims_list = []
    for t in range(4):
        scr = scrp.tile([128, 15, D], BF16_DT, tag="scr", name="scr")
        sims = simp.tile([128, 16], F32, tag="sims", name="sims")
        for j, (v, w_) in enumerate(_PAIRS):
            nc.vector.scalar_tensor_tensor(
                scr[:, j, :],
                zb_sb[t][:, v * D:(v + 1) * D], 1.0,
                zb_sb[t][:, w_ * D:(w_ + 1) * D],
                op0=mybir.AluOpType.mult, op1=mybir.AluOpType.mult,
                accum_out=sims[:, j:j + 1],
            )
        sims_list.append(sims)
    pexp_at = {16: 0, 18: 1, 20: 2, 22: 3}

    idx = 0
    for g in range(NGROUP):
        gtiles = stream[idx:idx + GTILES]
        idx += len(gtiles)
        w = len(gtiles) * 128
        ps = psp.tile([128, GCOLS], F32, tag="ps", name="ps")

        # merge consecutive w2 tiles of the same row within one PSUM bank
        runs = []  # (slot0, ntiles, v, src, base, col0, biased)
        for slot, (v, src, base, t) in enumerate(gtiles):
            col = base + t * 128
            if t == 0:
                runs.append([slot, 1, v, src, base, col, True])
                continue
            if runs:
                s0, n, pv, psrc, pbase, pcol, pb = runs[-1]
                if (not pb and pv == v and psrc == src and pbase == base
                        and pcol + n * 128 == col
                        and (s0 * 128) // 512 == (slot * 128) // 512):
                    runs[-1][1] += 1
                    continue
            runs.append([slot, 1, v, src, base, col, False])

        for (s0, n, v, src, base, col, biased) in runs:
            zt = zsb[src][v]
            nc.tensor.matmul(
                ps[:, s0 * 128:(s0 + n) * 128],
                zt[:, :, base:base + 128],
                zt[:, :, col:col + n * 128],
                start=True, stop=not biased,
                perf_mode=mybir.MatmulPerfMode.DoubleRow,
            )
            if biased:
                nc.tensor.matmul(
                    ps[:, s0 * 128:(s0 + 1) * 128],
                    bl[:, :], br[:, :],
                    start=False, stop=True,
                )

        esc = escp.tile([128, GCOLS], BF16_DT, tag="esc", name="esc")
        if g in POOL_SET:
            # Pool software pow: exp(2s) = (e^2)^s  (HW-verified ucode path);
            # the free-axis sum runs on the DVE (Pool cannot reduce free dims).
            escf = escp.tile([128, GCOLS], F32, tag="escf", name="escf")
            nc.gpsimd.tensor_tensor(escf[:, 0:w], e2t[:, 0:w], ps[:, 0:w],
                                    op=mybir.AluOpType.pow)
            nc.vector.tensor_reduce(
                stats[:, g:g + 1], escf[:, 0:w],
                axis=mybir.AxisListType.X, op=mybir.AluOpType.add,
            )
        else:
            nc.scalar.activation(
                esc[:, 0:w], ps[:, 0:w],
                mybir.ActivationFunctionType.Exp,
                bias=0.0, scale=2.0,
                accum_out=stats[:, g:g + 1],
            )
        if g in pexp_at:
            t = pexp_at[g]
            pexp = pexpp.tile([128, 16], BF16_DT, tag="pexp", name="pexp")
            nc.scalar.activation(
                pexp[:, 0:15], sims_list[t][:, 0:15],
                mybir.ActivationFunctionType.Exp,
                bias=0.0, scale=-2.0,
                accum_out=stats[:, POS_BASE + t: POS_BASE + t + 1],
            )


_NC_CACHE = None


def _get_nc() -> bass.Bass:
    global _NC_CACHE
    if _NC_CACHE is None:
        _NC_CACHE = _build_nc()
    return _NC_CACHE


def _prep_inputs(z: np.ndarray) -> list[dict[str, np.ndarray]]:
    z = np.asarray(z, dtype=np.float32)
    zn = z / np.linalg.norm(z, axis=-1, keepdims=True)          # [B, V, D]
    znT = np.ascontiguousarray(zn.transpose(1, 2, 0))           # [V, D, B]
    ext = np.concatenate([znT, znT[:, :, :EXT - B]], axis=2)    # [V, D, EXT]
    # [V, D, EXT] -> [V, 2, 128, EXT] -> [V, 128, 2, EXT] fp8
    ext8 = np.ascontiguousarray(
        ext.reshape(V, 2, 128, EXT).transpose(0, 2, 1, 3)).astype(FP8NP)
    in_maps = []
    for c in range(NCORE):
        lo0 = 2 * c * 128
        hi0 = (16 + 2 * c) * 128
        in_maps.append({
            "zlo": np.ascontiguousarray(ext8[:, :, :, lo0:lo0 + WIN]),
            "zhi": np.ascontiguousarray(ext8[:, :, :, hi0:hi0 + WIN]),
            "zb": np.ascontiguousarray(
                zn[512 * c:512 * (c + 1)].reshape(4, 128, V * D)
            ).astype(BF16),
        })
    return in_maps


def _host_reduce(stats_list) -> np.float32:
    e2 = float(np.exp(2.0))
    neg = 0.0
    P = 0.0
    for c in range(NCORE):
        st = np.asarray(stats_list[c], dtype=np.float64)
        neg += 2.0 * st[:, 0:NGROUP].sum()
        P += st[:, POS_BASE:POS_BASE + 4].sum()
    neg -= B * V * e2                      # analytic gram diagonal
    pos_sum = (2.0 * e2 * P + V * B) / (V * V)
    total = SCALE_LOSS * pos_sum + LAMBDA * neg / (B - 1)
    return np.float32(total)


def run(z: np.ndarray, trace: bool = False):
    nc = _get_nc()
    in_maps = _prep_inputs(z)
    res = run_bass_kernel_spmd(
        nc, in_maps, core_ids=list(range(NCORE)), trace=trace
    )
    stats_list = [res.results[c]["stats"] for c in range(NCORE)]
    return _host_reduce(stats_list), res


def kernel(z: np.ndarray) -> np.ndarray:
    loss, _ = run(z, trace=False)
    return np.asarray(loss, dtype=np.float32)

